# revision 31
# baseline (speedup 1.0000x reference)
"""Trainium2 Bass kernel: out = softmax(gelu_tanh(x @ W^T), axis=-1) + bias.

Full shapes: x [8192, 4096] f32, weight [4096, 4096] f32, bias [4096] f32.
Sharding: data-parallel over rows of x across 8 NeuronCores (1024 rows/core);
weight and bias replicated. Matmul runs in fp8e4m3 DoubleRow mode (157 TF/s,
2x bf16) with fp32 PSUM accumulation; x is pre-scaled by 16 and W by 64 so
both operands sit well inside e4m3's normal range, and the scales are undone
inside the ACT-engine epilogue. Gelu uses the exact tanh-approx constants of
the reference via Square/Tanh/Exp + Identity (all in the one `exp_and_others`
ACT table set -> exactly one ACT_TABLE_LOAD); softmax needs no max-subtraction
because gelu output is bounded (exp arg <= ~3.5).

Per-core structure (MC=1024 rows = 8 m-tiles of 128):
  x is fully SBUF-resident (32KB/partition); W streams through SBUF exactly
  once as 8 n-tiles of 512 cols in four chunks of 2. Chunk 0 runs j-outer
  (all m-tiles against w0 while w1 streams) so the PE never starves during
  the lead-in; later chunks run i-outer, accumulating two PSUM tiles per
  m-tile (16 DoubleRow matmuls of k=256 each) and fusing exp(gelu(v)) into
  the PSUM->SBUF epilogue with per-row partial sums accumulated by the ACT
  engine (ACT also does the A*v^2+1 affine via Identity; DVE does only the
  two PSUM-operand ops). In the FINAL chunk each m-tile's row sums complete
  as soon as its last n-tile drains: row-sum runs on ACT (Copy+accum_out),
  then DVE normalizes via tensor_scalar (4x mode, p*recip) + two
  tensor_tensor halves (2x mode, +bias; scalar_tensor_tensor has no fast
  DVE mode), overlapping the remaining m-tiles' matmuls. Output is written
  fp16 (halves out DMA; ~5e-4 added rounding error) and upcast on the host.

History: bf16 version 490-497us (bf16 PE roofline 78.6 TF/s); fp8 j-outer
302us (17us W-reload boundary gap + 40us serialized normalize tail); this
version 267us = ~14us lead-in + ~225us matmul stream (PE busy ~222.5us,
within 4% of the fp8 DoubleRow roofline -- the PE sustains ~2.3GHz) + ~27us
tail (last rows' epilogue drain + normalize + final DMA + fixed ~10us NEFF
semaphore drain). Error 1.14e-2 of absmax (fp8 operand quantization
dominated), within the 2e-2 gate; Frobenius rel err 5.8e-4.
"""

import sys

if "/opt/trn_rl_repo" not in sys.path:
    sys.path.insert(0, "/opt/trn_rl_repo")

import ml_dtypes
import numpy as np

import concourse.bass as bass
import concourse.tile as tile
from concourse import bacc, mybir
from concourse.bass_utils import run_bass_kernel_spmd

P = 128
GELU_A = 0.044715
GELU_C = 0.7978845608

# Full-problem constants (hardcoded; harness calls kernel() with these shapes)
FULL_M, FULL_K, FULL_N = 8192, 4096, 4096
NCORES = 8
MC = FULL_M // NCORES  # rows per core
KO = FULL_K // P       # 32 k-subtiles of 128
NT = 512               # n tile (columns per weight tile / psum bank)
NJ = FULL_N // NT      # 8 n-tiles
MT = MC // P           # 8 m-tiles of 128 rows
CHUNKS = ((0, 1), (2, 3), (4, 5), (6, 7))  # n-tile chunks of W; in the final
                                           # chunk each row normalizes as soon
                                           # as its last n-tile drains

W_SCALE = 64.0  # weight values ~U(-1/64,1/64) sit at e4m3's min-normal
                # boundary; scale into [-1,1] for the matmul.
X_SCALE = 16.0  # x ~N(0,1): scale past e4m3's subnormal region (max |16x|~88
                # stays well under e4m3's 448 max).
SCALE = W_SCALE * X_SCALE  # PSUM holds SCALE * v; undone in the epilogue


def build_nc():
    """Emit the per-core fp8 Bass program. Each core computes MC rows."""
    f32 = mybir.dt.float32
    f16 = mybir.dt.float16
    in_dt = mybir.dt.float8e4
    N = FULL_N

    nc = bacc.Bacc("TRN2", target_bir_lowering=False, debug=False)
    xt = nc.dram_tensor("xt", [MT, P, KO, P], in_dt, kind="ExternalInput").ap()
    wt = nc.dram_tensor("wt", [NJ, P, KO, NT], in_dt, kind="ExternalInput").ap()
    bias = nc.dram_tensor("bias", [P, N], f16, kind="ExternalInput").ap()
    out = nc.dram_tensor("out", [P, MT, N], f16, kind="ExternalOutput").ap()

    with tile.TileContext(nc) as tc:
        with (
            tc.tile_pool(name="const", bufs=1) as const_pool,
            tc.tile_pool(name="x", bufs=1) as x_pool,
            tc.tile_pool(name="w", bufs=4) as w_pool,
            tc.tile_pool(name="probs", bufs=1) as probs_pool,
            tc.tile_pool(name="tmp", bufs=2) as tmp_pool,
            tc.tile_pool(name="stat", bufs=1) as stat_pool,
            tc.tile_pool(name="stage", bufs=2) as stage_pool,
            tc.tile_pool(name="psum", bufs=8, space="PSUM") as psum_pool,
        ):
            bias_t = const_pool.tile([P, N], f16)
            xr = x_pool.tile([P, KO, MC], in_dt)
            probs = probs_pool.tile([P, MT, N], f16)
            sums = stat_pool.tile([P, MT * NJ], f32, tag="sums")
            ssum = stat_pool.tile([P, MT], f32, tag="ssum")
            recips = stat_pool.tile([P, MT], f32, tag="recips")

            # DMA emission order is DMA-queue FIFO priority: x m-tile 0 first
            # (the first matmul's stationary), then w0's k-chunks back-to-back
            # (its consumption is k-ascending), then w1, then the rest of x.
            # Chunk 1's w tiles follow into the two spare w bufs; chunks 2/3
            # are emitted after earlier chunks' compute (their buffer-free
            # semaphores gate them, and nothing later on the DGE queue is
            # needed sooner).
            wtiles = {}
            for j in CHUNKS[0]:
                wtiles[j] = w_pool.tile([P, KO, NT], in_dt, tag="w", name=f"w{j}")
            WKCH = 4
            KW = KO // WKCH
            nc.gpsimd.dma_start(xr[:, :, 0:P], xt[0])
            for c in range(WKCH):
                nc.gpsimd.dma_start(
                    wtiles[CHUNKS[0][0]][:, c * KW : (c + 1) * KW, :],
                    wt[CHUNKS[0][0], :, c * KW : (c + 1) * KW, :],
                )
            # chunk 0 runs j-outer, so all x m-chunks are consumed against w0
            # first; stream them ahead of w1.
            for c in range(1, MT):
                nc.gpsimd.dma_start(xr[:, :, c * P : (c + 1) * P], xt[c])
            for c in range(WKCH):
                nc.gpsimd.dma_start(
                    wtiles[CHUNKS[0][1]][:, c * KW : (c + 1) * KW, :],
                    wt[CHUNKS[0][1], :, c * KW : (c + 1) * KW, :],
                )
            nc.gpsimd.dma_start(bias_t[:], bias[:])
            for j in CHUNKS[1]:
                wtiles[j] = w_pool.tile([P, KO, NT], in_dt, tag="w", name=f"w{j}")
                nc.gpsimd.dma_start(wtiles[j][:], wt[j])

            def mm_tile(i, j):
                ps = psum_pool.tile([P, NT], f32, name="ps", tag="ps")
                for k in range(0, KO, 2):
                    nc.tensor.matmul(
                        ps[:],
                        xr[:, k : k + 2, i * P : (i + 1) * P],
                        wtiles[j][:, k : k + 2, :],
                        start=(k == 0),
                        stop=(k == KO - 2),
                        perf_mode=mybir.MatmulPerfMode.DoubleRow,
                    )
                return ps

            def epilogue(i, j, ps):
                # p = exp(gelu(v)), gelu = 0.5*v*(1+tanh(C*(v+A*v^3)))
                # with ps = SCALE*v. Square/Identity/Tanh/Exp all live in
                # the exp_and_others table set (no table reloads); ACT
                # absorbs the A*v^2+1 affine so DVE only does the two
                # PSUM-operand ops the ACT engine cannot.
                v2 = tmp_pool.tile([P, NT], f16, tag="v2", name="v2")
                nc.scalar.activation(
                    v2[:], ps[:], mybir.ActivationFunctionType.Square,
                    bias=0.0, scale=1.0 / SCALE,
                )
                t1 = tmp_pool.tile([P, NT], f16, tag="t1", name="t1")
                nc.scalar.activation(
                    t1[:], v2[:], mybir.ActivationFunctionType.Identity,
                    bias=1.0, scale=GELU_A,
                )
                t2 = tmp_pool.tile([P, NT], f16, tag="t2", name="t2")
                nc.vector.tensor_mul(t2[:], ps[:], t1[:])
                th = tmp_pool.tile([P, NT], f16, tag="th", name="th")
                nc.scalar.activation(
                    th[:], t2[:], mybir.ActivationFunctionType.Tanh,
                    bias=0.0, scale=GELU_C / SCALE,
                )
                g2 = tmp_pool.tile([P, NT], f32, tag="g2", name="g2")
                nc.vector.scalar_tensor_tensor(
                    g2[:], th[:], 1.0, ps[:],
                    mybir.AluOpType.add, mybir.AluOpType.mult,
                )
                sidx = i * NJ + j
                nc.scalar.activation(
                    probs[:, i, j * NT : (j + 1) * NT], g2[:],
                    mybir.ActivationFunctionType.Exp,
                    bias=0.0, scale=0.5 / SCALE,
                    accum_out=sums[:, sidx : sidx + 1],
                )

            NG = NJ
            last_ci = len(CHUNKS) - 1
            for ci, chunk in enumerate(CHUNKS):
                if ci == 0:
                    # j-outer for the first chunk: all 8 m-tiles run against
                    # w0 while w1 is still streaming in, so the PE never
                    # starves during the lead-in.
                    for j in chunk:
                        for i in range(MT):
                            epilogue(i, j, mm_tile(i, j))
                    for j in CHUNKS[2]:
                        wtiles[j] = w_pool.tile(
                            [P, KO, NT], in_dt, tag="w", name=f"w{j}"
                        )
                        nc.gpsimd.dma_start(wtiles[j][:], wt[j])
                    continue
                for i in range(MT):
                    pss = []
                    for j in chunk:
                        pss.append((j, mm_tile(i, j)))
                    for j, ps in pss:
                        epilogue(i, j, ps)
                    if ci == last_ci:
                        # Row i's sums are complete: normalize + bias + store
                        # now, overlapping m-tiles i+1..7's matmuls.
                        # Row-sum of the partials runs on ACT (Copy with
                        # accum_out) to keep it off the hotter DVE queue.
                        # scalar_tensor_tensor has no fast DVE mode, so split:
                        # tensor_scalar (4x mode on packed fp16) for p*recip,
                        # then tensor_tensor halves (2x mode) for +bias.
                        junk = stat_pool.tile([P, NG], f32, tag="junk")
                        nc.scalar.activation(
                            junk[:],
                            sums[:, i * NG : (i + 1) * NG],
                            mybir.ActivationFunctionType.Copy,
                            accum_out=ssum[:, i : i + 1],
                        )
                        nc.vector.reciprocal(
                            recips[:, i : i + 1], ssum[:, i : i + 1]
                        )
                        st = stage_pool.tile([P, N], f16, tag="st", bufs=1)
                        nc.vector.tensor_scalar(
                            st[:],
                            probs[:, i, :],
                            recips[:, i : i + 1],
                            None,
                            mybir.AluOpType.mult,
                        )
                        NH = N // 2
                        for h in range(2):
                            st2 = stage_pool.tile([P, NH], f16, tag="st2")
                            nc.vector.tensor_tensor(
                                st2[:],
                                st[:, h * NH : (h + 1) * NH],
                                bias_t[:, h * NH : (h + 1) * NH],
                                mybir.AluOpType.add,
                            )
                            nc.gpsimd.dma_start(
                                out[:, i, h * NH : (h + 1) * NH], st2[:]
                            )
                # Chunks 2+: w DMAs emitted after the chunk two back's compute
                # so their buffer-free waits resolve in order.
                if ci + 2 <= last_ci:
                    for j in CHUNKS[ci + 2]:
                        wtiles[j] = w_pool.tile(
                            [P, KO, NT], in_dt, tag="w", name=f"w{j}"
                        )
                        nc.gpsimd.dma_start(wtiles[j][:], wt[j])
    nc.compile()
    return nc


def pack_inputs(x, weight, bias):
    """Host-side shard + pack into the DMA-friendly layouts the kernel expects."""
    M, K = x.shape
    N = weight.shape[0]
    fp8 = ml_dtypes.float8_e4m3
    ncores = M // MC
    # wt[j, p, ko, n] = W_SCALE * weight[j*NT+n, ko*P+p]
    wt = np.ascontiguousarray(
        (weight * W_SCALE).astype(fp8).reshape(NJ, NT, KO, P).transpose(0, 3, 2, 1)
    )
    bias_b = np.ascontiguousarray(
        np.broadcast_to(bias.astype(np.float16)[None, :], (P, N))
    )
    in_maps = []
    for c in range(ncores):
        xs = (x[c * MC : (c + 1) * MC] * X_SCALE).astype(fp8)
        # xt[i, p, ko, m] = X_SCALE * x_core[i*P+m, ko*P+p]  (m-tile-major)
        xtc = np.ascontiguousarray(xs.reshape(MT, P, KO, P).transpose(0, 3, 2, 1))
        in_maps.append({"xt": xtc, "wt": wt, "bias": bias_b})
    return in_maps


def unpack_outputs(results):
    outs = []
    for res in results:
        o = np.asarray(res["out"]).astype(np.float32)  # [P, MT, N] bf16
        outs.append(o.transpose(1, 0, 2).reshape(MC, FULL_N))
    return np.concatenate(outs, axis=0)


_CACHE = {}


def _get_nc():
    if "nc" not in _CACHE:
        _CACHE["nc"] = build_nc()
    return _CACHE["nc"]


def _ensure_trace_env():
    """The agent image's antenv lacks axon_hooks, so NTFF tracing silently
    degrades. Register the ctypes-based hook ourselves, and neuter the S3
    artifact upload (no bucket access here)."""
    try:
        from antenv.axon_hooks import get_axon_ntff_profile_hook  # noqa: F401
    except ImportError:
        import types

        import antenv
        from trn_agent_boot.trn_boot import _ntff_profile_via_ctypes

        mod = types.ModuleType("antenv.axon_hooks")
        state = {"hook": _ntff_profile_via_ctypes("/opt/axon/libaxon_pjrt.so")}
        mod.set_axon_ntff_profile_hook = lambda h: state.__setitem__("hook", h)
        mod.get_axon_ntff_profile_hook = lambda: state["hook"]
        sys.modules["antenv.axon_hooks"] = mod
        antenv.axon_hooks = mod
    import concourse.bass_utils as bu

    bu.upload_artifacts = lambda tmpdir: f"local://{tmpdir}"


def kernel(x, weight, bias, trace=False):
    if trace:
        _ensure_trace_env()
    nc = _get_nc()
    in_maps = pack_inputs(
        np.asarray(x, dtype=np.float32),
        np.asarray(weight, dtype=np.float32),
        np.asarray(bias, dtype=np.float32),
    )
    res = run_bass_kernel_spmd(nc, in_maps, core_ids=list(range(NCORES)), trace=trace)
    out = unpack_outputs(res.results)
    if trace:
        return out, res
    return out
